# revision 24
# baseline (speedup 1.0000x reference)
"""Trainium2 Bass kernel: 3-layer GAT (nn_GAT_62182536511748).

Strategy (8 NeuronCores, SPMD, fp16 single-row gather), v3:
  - Nodes sharded contiguously across cores (6250 valid/core, padded to
    6272 = 49*128). dst == repeat(arange(N), 16): 16 in-edges per node.
  - Per layer each core computes feat = x_shard @ W (fp16 PE) plus the
    attention dots el/er via an extended weight matrix. Bias is folded
    into the table rows (softmax weights sum to 1, so
    sum_k alpha_k (feat_k + b) == out + b exactly).
  - Table rows are SINGLE-node [feat|el] fp16 at a 512B (L1/L2) / 256B
    (L3) stride. int16 gather indices only span +-32k, so indices are
    stored SHIFTED by NT/2 and the gather base points at the table
    middle - negative indices address the lower half (verified on HW).
  - The table is AllGather'd in three region chunks (fired early, with
    a 3-group lag so the gpsimd gather stream keeps its lookahead).
  - Edge phase per 128-node group: two 1024-index dma_gather
    instructions fetch one 264B/82B row per edge straight into
    node-slot position; fp16 multiply + fp16 pairwise add-tree
    aggregate; per-node softmax scale applied after the reduction.
  - The NEXT layer's feat phase (PE transpose + matmul) is interleaved
    into the current layer's edge loop; h never round-trips DRAM.
  - Scalar engine runs Exp only (leaky/relu/copies on DVE; plain
    tensor_scalar is avoided - it hits a slow ucode path - in favor of
    scalar_tensor_tensor). Final log-softmax Ln is one batched tail op.
"""

import os
import numpy as np

# ---- fixed problem dims -------------------------------------------------
N = 50000
DEG = 16
IN = 256
HID = 32
HEAD = 4
OUT = 40
HH = HID * HEAD  # 128
NEG_SLOPE = 0.2
NCORES = 8
NV = N // NCORES          # 6250 valid nodes per core

SUB12 = HH + 4            # 132 fp16 payload per node row (layers 1/2)
ROW12 = 256               # fp16 row stride (512B)
SUB3 = OUT + 1            # 41
ROW3 = 128                # 256B stride
NIDX = 1024               # indices per dma_gather (2048 crashes the ring)
GB = (0, 25, 45, 49)      # collective region boundaries in groups
LAG = 3                   # groups of slack before firing a collective
SHIFT = 17408             # gather base row: idx = row-SHIFT in [-17408,32767]

_PROGRAM_CACHE = {}
LAST_RESULTS = None


def _dma_gather_raw(nc, mybir, out_ap, in_ap, idxs_ap, num_idxs, elem_size,
                    elem_step, queue_num=0):
    """dma_gather minus the over-strict elem%256B assert (stride must still
    be a 256B multiple; verified on HW with 528B/264B/164B/82B elems)."""
    eng = nc.gpsimd
    stride_bytes = elem_step * mybir.dt.size(in_ap.dtype)
    assert stride_bytes % 256 == 0 and stride_bytes // 256 < 256
    _in_ap = eng.lower_ap_dma(in_ap, for_custom_bir_dma=True)
    _idxs_ap = eng.lower_ap(idxs_ap)
    _out_ap = eng.lower_ap(out_ap)
    return eng.add_instruction(
        mybir.InstDMAGatherAnt(
            name=nc.get_next_instruction_name(),
            ins=[*_in_ap, _idxs_ap,
                 eng.lower_val_access(eng.to_reg(num_idxs))],
            outs=[_out_ap],
            transpose=False, num_idxs=num_idxs, elem_size=elem_size,
            stride_bytes_256=stride_bytes // 256, gen_mode=0,
            single_packet=True, queue_num=queue_num,
            sbuf_tokens_per_rank=0, sbuf_free_dim_per_rank=0,
            sbuf_free_dim_pad_per_rank=0, sbuf_byte_offset=0,
        ))


# ========================================================================
# device program
# ========================================================================
def _build_program(ncores: int, ns_pad: int):
    from concourse import bass, mybir, tile, bacc
    from concourse.masks import make_identity
    from concourse.library_config import mlp

    f32 = mybir.dt.float32
    f16 = mybir.dt.float16
    i16 = mybir.dt.int16
    AX = mybir.AxisListType
    OPT = mybir.AluOpType
    AF = mybir.ActivationFunctionType

    G = ns_pad // 128
    NT = ncores * ns_pad          # table rows (50176)
    LRB = [128 * b for b in GB]   # local row bounds  [0,3200,5760,6272]
    GFB = [ncores * b for b in LRB]

    nc = bacc.Bacc(
        "TRN2", target_bir_lowering=False, debug=False,
        enable_asserts=False, num_devices=ncores, num_swdge_queues=4)

    # ---- kernel I/O ----
    x0t_d = nc.dram_tensor("x0t", [128, 2 * ns_pad], f16,
                           kind="ExternalInput").ap()
    idx_d = nc.dram_tensor("idx", [128, G * 128], i16,
                           kind="ExternalInput").ap()
    w1_d = nc.dram_tensor("w1", [IN, HH + 2 * HEAD], f16,
                          kind="ExternalInput").ap()
    wh_d = nc.dram_tensor("wh", [HH, HH + 2], f16, kind="ExternalInput").ap()
    w2_d = nc.dram_tensor("w2", [HH, OUT + 2], f16,
                          kind="ExternalInput").ap()
    be1_d = nc.dram_tensor("be1", [128, SUB12], f32, kind="ExternalInput").ap()
    beh_d = nc.dram_tensor("beh", [128, HH + 1], f32,
                           kind="ExternalInput").ap()
    be2_d = nc.dram_tensor("be2", [128, SUB3], f32, kind="ExternalInput").ap()
    out_d = nc.dram_tensor("out", [ns_pad, OUT], f32,
                           kind="ExternalOutput").ap()

    shared = "Shared" if ncores > 4 else "Local"
    # gst: tight per-core shard rows; gff: tight allgathered table;
    # gf: 256B/512B-row-stride table the gather reads (local re-stride).
    gs1_d = nc.dram_tensor("gs1", [ns_pad, SUB12], f16).ap()
    gs2_d = nc.dram_tensor("gs2", [ns_pad, SUB12], f16).ap()
    gs3_d = nc.dram_tensor("gs3", [ns_pad, SUB3], f16).ap()
    gg1_d = nc.dram_tensor("gg1", [NT, SUB12], f16, addr_space=shared).ap()
    gg2_d = nc.dram_tensor("gg2", [NT, SUB12], f16, addr_space=shared).ap()
    gg3_d = nc.dram_tensor("gg3", [NT, SUB3], f16, addr_space=shared).ap()
    gf1_d = nc.dram_tensor("gf1", [NT, ROW12], f16).ap()
    gf2_d = nc.dram_tensor("gf2", [NT, ROW12], f16).ap()
    gf3_d = nc.dram_tensor("gf3", [NT, ROW3], f16).ap()

    rgroups = [list(range(ncores))]

    with tile.TileContext(nc) as tc:
        with (
            tc.tile_pool(name="const", bufs=1) as cp,
            tc.tile_pool(name="feat", bufs=3) as fp,
            tc.tile_pool(name="edge", bufs=3) as ep,
            tc.tile_pool(name="psum", bufs=2, space="PSUM") as pp,
        ):
            nc.gpsimd.load_library(mlp)
            ident32 = cp.tile([128, 128], f32)
            make_identity(nc, ident32[:])
            ident16 = cp.tile([128, 128], f16)
            nc.vector.tensor_copy(ident16[:], ident32[:])
            zeros = cp.tile([128, HH], f32)
            nc.vector.memset(zeros[:], 0.0)
            idx_sb = cp.tile([128, G * 128], i16)
            nc.sync.dma_start(out=idx_sb[:], in_=idx_d[:, :])
            w1a = cp.tile([128, HH + 2 * HEAD], f16)
            w1b = cp.tile([128, HH + 2 * HEAD], f16)
            nc.sync.dma_start(out=w1a[:], in_=w1_d[0:128, :])
            nc.sync.dma_start(out=w1b[:], in_=w1_d[128:256, :])
            wh_sb = cp.tile([128, HH + 2], f16)
            nc.sync.dma_start(out=wh_sb[:], in_=wh_d[:, :])
            w2_sb = cp.tile([128, OUT + 2], f16)
            nc.sync.dma_start(out=w2_sb[:], in_=w2_d[:, :])
            be1 = cp.tile([128, SUB12], f32)
            nc.sync.dma_start(out=be1[:], in_=be1_d[:, :])
            beh = cp.tile([128, HH + 1], f32)
            nc.sync.dma_start(out=beh[:], in_=beh_d[:, :])
            be2 = cp.tile([128, SUB3], f32)
            nc.sync.dma_start(out=be2[:], in_=be2_d[:, :])
            er1 = cp.tile([128, G * HEAD], f32)
            er2 = cp.tile([128, G], f32)
            er3 = cp.tile([128, G], f32)
            ht3 = cp.tile([128, G * OUT], f32)
            nm3 = cp.tile([128, G], f32)
            s3 = cp.tile([128, G], f32)
            ls3 = cp.tile([128, G], f32)

            def region_cc(gs_d, gg_d, gf_d, sub, r):
                a, b = LRB[r], LRB[r + 1]
                nc.gpsimd.collective_compute(
                    "AllGather", OPT.bypass, replica_groups=rgroups,
                    ins=[gs_d[a:b, :]],
                    outs=[gg_d[GFB[r]:GFB[r + 1], :]])
                # local re-stride to the 256B-multiple row pitch the
                # gather needs (overlaps the next region's collective)
                nc.sync.dma_start(
                    out=gf_d[GFB[r]:GFB[r + 1], 0:sub],
                    in_=gg_d[GFB[r]:GFB[r + 1], :])

            def maybe_cc(gs_d, gg_d, gf_d, sub, g, lags):
                for r in range(3):
                    if g == min(GB[r + 1] - 1 + lags[r], G - 1):
                        region_cc(gs_d, gg_d, gf_d, sub, r)

            def fence(gf_d, tag):
                """Order the gpsimd gather stream after all three region
                collectives: the raw gather only declares the upper half of
                the table as its input (the base is mid-table for signed
                idx), so collectives writing the lower half are otherwise
                unordered against it. Probe one row per region via sync-DMA,
                then touch the probe tile from gpsimd."""
                probe = cp.tile([128, ROW12], f16, name=f"probe_{tag}")
                for r in range(3):
                    w = min(ROW12, gf_d.shape[1])
                    nc.sync.dma_start(out=probe[r:r + 1, 0:w],
                                      in_=gf_d[GFB[r]:GFB[r] + 1, 0:w])
                scr = cp.tile([128, 8], f16, name=f"scr_{tag}")
                nc.gpsimd.tensor_copy(scr[0:3, :], probe[0:3, 0:8])

            def gathers(gf_d, sub, row, g):
                big = ep.tile([128, DEG * sub], f16, tag=f"big{row}", bufs=8,
                              name=f"big_{row}_{g}")
                for h in range(2):
                    _dma_gather_raw(
                        nc, mybir,
                        big[:, h * 8 * sub:(h + 1) * 8 * sub],
                        gf_d[SHIFT:NT, 0:sub],
                        idx_sb[:, g * 128 + h * 64:g * 128 + (h + 1) * 64],
                        NIDX, sub, row, queue_num=(2 * g + h) % 4)
                return big[:].rearrange("p (k r) -> p k r", r=sub)

            # ---------------- L1 feat ----------------
            for g in range(G):
                c0, c1 = g * 128, (g + 1) * 128
                xt = fp.tile([128, 256], f16, tag="xt", bufs=4,
                             name=f"A_x{g}")
                xv = x0t_d[:, :].rearrange("p (c n) -> p c n", n=ns_pad)
                nc.sync.dma_start(
                    out=xt[:].rearrange("p (c n) -> p c n", c=2),
                    in_=xv[:, :, c0:c1])
                fps = pp.tile([128, HH + 2 * HEAD], f32, tag="fps1",
                              name=f"A_fps{g}")
                nc.tensor.matmul(fps[:], lhsT=xt[:, 0:128], rhs=w1a[:],
                                 start=True, stop=False)
                nc.tensor.matmul(fps[:], lhsT=xt[:, 128:256], rhs=w1b[:],
                                 start=False, stop=True)
                grow = fp.tile([128, SUB12], f16, tag="grow",
                               name=f"A_grow{g}")
                nc.vector.tensor_tensor(out=grow[:], in0=fps[:, 0:SUB12],
                                        in1=be1[:], op=OPT.add)
                nc.vector.tensor_copy(er1[:, g * HEAD:(g + 1) * HEAD],
                                      fps[:, SUB12:SUB12 + HEAD])
                nc.sync.dma_start(out=gs1_d[c0:c1, :], in_=grow[:])
                maybe_cc(gs1_d, gg1_d, gf1_d, SUB12, g, (0, 0, 0))

            # ---------------- L1 edge + L2 feat ----------------
            fence(gf1_d, "1")
            for g in range(G):
                bv = gathers(gf1_d, SUB12, ROW12, g)
                feat_e = bv[:, :, 0:HH]              # [128,16,128] f16
                el_e = bv[:, :, HH:HH + HEAD]        # [128,16,4] f16
                f_all = ep.tile([128, DEG * HH], f16, tag="f_all",
                                name=f"B_fa{g}")
                e_t = ep.tile([128, DEG * HEAD], f32, tag="e_t",
                              name=f"B_et{g}")
                erv = (er1[:, g * HEAD:(g + 1) * HEAD]
                       .unsqueeze(1).to_broadcast((128, DEG, HEAD)))
                nc.vector.tensor_tensor(
                    out=e_t[:].rearrange("p (k h) -> p k h", h=HEAD),
                    in0=el_e, in1=erv, op=OPT.add)
                e2 = ep.tile([128, DEG * HEAD], f32, tag="e2",
                             name=f"B_e2{g}")
                nc.vector.scalar_tensor_tensor(
                    out=e2[:], in0=e_t[:], scalar=NEG_SLOPE, in1=e_t[:],
                    op0=OPT.mult, op1=OPT.max)
                ex = ep.tile([128, DEG * HEAD], f16, tag="ex",
                             name=f"B_ex{g}")
                nc.scalar.activation(out=ex[:], in_=e2[:], func=AF.Exp)
                den = ep.tile([128, HEAD], f32, tag="den", name=f"B_den{g}")
                nc.vector.tensor_reduce(
                    out=den[:],
                    in_=ex[:].rearrange("p (k h) -> p h k", h=HEAD),
                    axis=AX.X, op=OPT.add)
                inv = ep.tile([128, HEAD], f32, tag="inv", name=f"B_inv{g}")
                nc.vector.reciprocal(inv[:], den[:])
                featv = feat_e.rearrange("p k (h d) -> p k h d", h=HEAD)
                exv = (ex[:].rearrange("p (k h) -> p k h", h=HEAD)
                       .unsqueeze(3).to_broadcast((128, DEG, HEAD, HID)))
                nc.vector.tensor_tensor(
                    out=f_all[:].rearrange("p (k h d) -> p k h d",
                                           k=DEG, h=HEAD),
                    in0=featv, in1=exv, op=OPT.mult)
                for wdt in (8 * HH, 4 * HH, 2 * HH):
                    nc.vector.tensor_tensor(
                        out=f_all[:, 0:wdt], in0=f_all[:, 0:wdt],
                        in1=f_all[:, wdt:2 * wdt], op=OPT.add)
                u = ep.tile([128, HH], f32, tag="u", name=f"B_u{g}")
                nc.vector.tensor_tensor(out=u[:], in0=f_all[:, 0:HH],
                                        in1=f_all[:, HH:2 * HH], op=OPT.add)
                t1 = ep.tile([128, HH], f32, tag="t1", name=f"B_t1{g}")
                invv = inv[:].unsqueeze(2).to_broadcast((128, HEAD, HID))
                nc.vector.tensor_tensor(
                    out=t1[:].rearrange("p (h d) -> p h d", h=HEAD),
                    in0=u[:].rearrange("p (h d) -> p h d", h=HEAD),
                    in1=invv, op=OPT.mult)
                h1 = ep.tile([128, HH], f16, tag="h1", name=f"B_h1{g}")
                nc.vector.scalar_tensor_tensor(
                    out=h1[:], in0=t1[:], scalar=0.0, in1=zeros[:],
                    op0=OPT.max, op1=OPT.max)
                # ---- L2 feat for this group ----
                hT_ps = pp.tile([128, 128], f16, tag="hT", name=f"B_hT{g}")
                nc.tensor.transpose(hT_ps[:], h1[:], ident16[:])
                hT = fp.tile([128, 128], f16, tag="hTs", name=f"B_hTs{g}")
                nc.vector.tensor_copy(hT[:], hT_ps[:])
                fps2 = pp.tile([128, HH + 2], f32, tag="fps2",
                               name=f"B_fps2{g}")
                nc.tensor.matmul(fps2[:], lhsT=hT[:], rhs=wh_sb[:],
                                 start=True, stop=True)
                grow2 = fp.tile([128, HH + 1], f16, tag="grow",
                                name=f"B_grow2{g}")
                nc.vector.tensor_tensor(out=grow2[:],
                                        in0=fps2[:, 0:HH + 1],
                                        in1=beh[:], op=OPT.add)
                nc.vector.tensor_copy(er2[:, g:g + 1],
                                      fps2[:, HH + 1:HH + 2])
                nc.sync.dma_start(out=gs2_d[g * 128:(g + 1) * 128, 0:HH + 1],
                                  in_=grow2[:])
                maybe_cc(gs2_d, gg2_d, gf2_d, SUB12, g, (LAG, 1, 0))

            # ---------------- L2 edge + L3 feat ----------------
            fence(gf2_d, "2")
            for g in range(G):
                bv = gathers(gf2_d, SUB12, ROW12, g)
                feat_e = bv[:, :, 0:HH]
                el_e = bv[:, :, HH:HH + 1].rearrange("p k o -> p (k o)")
                f_all = ep.tile([128, DEG * HH], f16, tag="f_all",
                                name=f"C_fa{g}")
                e_t = ep.tile([128, DEG], f32, tag="e_t", name=f"C_et{g}")
                nc.vector.scalar_tensor_tensor(
                    out=e_t[:], in0=el_e, scalar=er2[:, g:g + 1],
                    in1=zeros[:, 0:DEG], op0=OPT.add, op1=OPT.add)
                e2 = ep.tile([128, DEG], f32, tag="e2", name=f"C_e2{g}")
                nc.vector.scalar_tensor_tensor(
                    out=e2[:], in0=e_t[:], scalar=NEG_SLOPE, in1=e_t[:],
                    op0=OPT.mult, op1=OPT.max)
                ex = ep.tile([128, DEG], f16, tag="ex", name=f"C_ex{g}")
                den = ep.tile([128, 1], f32, tag="den", name=f"C_den{g}")
                nc.scalar.activation(out=ex[:], in_=e2[:], func=AF.Exp,
                                     accum_out=den[:])
                inv = ep.tile([128, 1], f32, tag="inv", name=f"C_inv{g}")
                nc.vector.reciprocal(inv[:], den[:])
                exv = ex[:].unsqueeze(2).to_broadcast((128, DEG, HH))
                nc.vector.tensor_tensor(
                    out=f_all[:].rearrange("p (k d) -> p k d", k=DEG),
                    in0=feat_e, in1=exv, op=OPT.mult)
                for wdt in (8 * HH, 4 * HH, 2 * HH):
                    nc.vector.tensor_tensor(
                        out=f_all[:, 0:wdt], in0=f_all[:, 0:wdt],
                        in1=f_all[:, wdt:2 * wdt], op=OPT.add)
                u = ep.tile([128, HH], f32, tag="u", name=f"C_u{g}")
                nc.vector.tensor_tensor(out=u[:], in0=f_all[:, 0:HH],
                                        in1=f_all[:, HH:2 * HH], op=OPT.add)
                h2 = ep.tile([128, HH], f16, tag="h1", name=f"C_h2{g}")
                nc.vector.scalar_tensor_tensor(
                    out=h2[:], in0=u[:], scalar=inv[:, 0:1], in1=zeros[:],
                    op0=OPT.mult, op1=OPT.max)
                # ---- L3 feat for this group ----
                hT_ps = pp.tile([128, 128], f16, tag="hT", name=f"C_hT{g}")
                nc.tensor.transpose(hT_ps[:], h2[:], ident16[:])
                hT = fp.tile([128, 128], f16, tag="hTs", name=f"C_hTs{g}")
                nc.vector.tensor_copy(hT[:], hT_ps[:])
                fps3 = pp.tile([128, OUT + 2], f32, tag="fps3",
                               name=f"C_fps3{g}")
                nc.tensor.matmul(fps3[:], lhsT=hT[:], rhs=w2_sb[:],
                                 start=True, stop=True)
                grow3 = fp.tile([128, SUB3], f16, tag="grow3",
                                name=f"C_grow3{g}")
                nc.vector.tensor_tensor(out=grow3[:], in0=fps3[:, 0:SUB3],
                                        in1=be2[:], op=OPT.add)
                nc.vector.tensor_copy(er3[:, g:g + 1],
                                      fps3[:, SUB3:SUB3 + 1])
                nc.sync.dma_start(out=gs3_d[g * 128:(g + 1) * 128, :],
                                  in_=grow3[:])
                maybe_cc(gs3_d, gg3_d, gf3_d, SUB3, g, (LAG, 1, 0))

            # ---------------- L3 edge ----------------
            fence(gf3_d, "3")
            for g in range(G):
                bv = gathers(gf3_d, SUB3, ROW3, g)
                feat_e = bv[:, :, 0:OUT]
                el_e = bv[:, :, OUT:OUT + 1].rearrange("p k o -> p (k o)")
                e_t = ep.tile([128, DEG], f32, tag="e_t", name=f"D_et{g}")
                nc.vector.scalar_tensor_tensor(
                    out=e_t[:], in0=el_e, scalar=er3[:, g:g + 1],
                    in1=zeros[:, 0:DEG], op0=OPT.add, op1=OPT.add)
                e2 = ep.tile([128, DEG], f32, tag="e2", name=f"D_e2{g}")
                nc.vector.scalar_tensor_tensor(
                    out=e2[:], in0=e_t[:], scalar=NEG_SLOPE, in1=e_t[:],
                    op0=OPT.mult, op1=OPT.max)
                ex = ep.tile([128, DEG], f16, tag="ex", name=f"D_ex{g}")
                den = ep.tile([128, 1], f32, tag="den", name=f"D_den{g}")
                nc.scalar.activation(out=ex[:], in_=e2[:], func=AF.Exp,
                                     accum_out=den[:])
                inv = ep.tile([128, 1], f32, tag="inv", name=f"D_inv{g}")
                nc.vector.reciprocal(inv[:], den[:])
                f_all = ep.tile([128, DEG * OUT], f16, tag="fa3",
                                name=f"D_fa{g}")
                exv = ex[:].unsqueeze(2).to_broadcast((128, DEG, OUT))
                nc.vector.tensor_tensor(
                    out=f_all[:].rearrange("p (k d) -> p k d", k=DEG),
                    in0=feat_e, in1=exv, op=OPT.mult)
                for wdt in (8 * OUT, 4 * OUT, 2 * OUT):
                    nc.vector.tensor_tensor(
                        out=f_all[:, 0:wdt], in0=f_all[:, 0:wdt],
                        in1=f_all[:, wdt:2 * wdt], op=OPT.add)
                u = ep.tile([128, OUT], f32, tag="u3", name=f"D_u{g}")
                nc.vector.tensor_tensor(out=u[:], in0=f_all[:, 0:OUT],
                                        in1=f_all[:, OUT:2 * OUT], op=OPT.add)
                htc = ht3[:, g * OUT:(g + 1) * OUT]
                nc.vector.scalar_tensor_tensor(
                    out=htc, in0=u[:], scalar=inv[:, 0:1],
                    in1=zeros[:, 0:OUT], op0=OPT.mult, op1=OPT.add)
                nc.vector.reduce_max(out=nm3[:, g:g + 1], in_=htc,
                                     axis=AX.X, negate=True)
                exf = ep.tile([128, OUT], f16, tag="exf", name=f"D_exf{g}")
                nc.scalar.activation(out=exf[:], in_=htc, func=AF.Exp,
                                     bias=nm3[:, g:g + 1],
                                     accum_out=s3[:, g:g + 1])

            # ---------------- log-softmax tail ----------------
            nc.scalar.activation(out=ls3[:], in_=s3[:], func=AF.Ln)
            for g in range(G):
                o_t = ep.tile([128, OUT], f32, tag="o_t", name=f"E_o{g}")
                nc.vector.scalar_tensor_tensor(
                    out=o_t[:], in0=ht3[:, g * OUT:(g + 1) * OUT],
                    scalar=nm3[:, g:g + 1],
                    in1=ls3[:, g:g + 1].to_broadcast((128, OUT)),
                    op0=OPT.add, op1=OPT.subtract)
                nc.sync.dma_start(out=out_d[g * 128:(g + 1) * 128, :],
                                  in_=o_t[:])

    nc.compile()
    return nc


# ========================================================================
# host side
# ========================================================================
def _get_program(ncores, ns_pad):
    key = (ncores, ns_pad)
    if key not in _PROGRAM_CACHE:
        _PROGRAM_CACHE[key] = _build_program(ncores, ns_pad)
    return _PROGRAM_CACHE[key]


def _numpy_fallback(feats, src, dst, W1, al1, ar1, b1, Wh, alh, arh, bh,
                    W2, al2, ar2, b2):
    n = feats.shape[0]

    def gat(x, W, al, ar, b):
        Hh, Dd = al.shape
        feat = (x @ W).reshape(n, Hh, Dd)
        el = (feat * al).sum(-1)
        er = (feat * ar).sum(-1)
        e = el[src] + er[dst]
        e = np.where(e > 0, e, NEG_SLOPE * e).astype(np.float32)
        emax = np.full((n, Hh), -np.inf, np.float32)
        np.maximum.at(emax, dst, e)
        ex = np.exp(e - emax[dst])
        den = np.zeros((n, Hh), np.float32)
        np.add.at(den, dst, ex)
        alpha = ex / den[dst]
        out = np.zeros((n, Hh, Dd), np.float32)
        np.add.at(out, dst, feat[src] * alpha[..., None])
        return out + b.reshape(1, Hh, Dd)

    h = np.maximum(gat(feats, W1, al1, ar1, b1).reshape(n, HH), 0.0)
    h = np.maximum(gat(h, Wh, alh, arh, bh).mean(1), 0.0)
    h = gat(h, W2, al2, ar2, b2).mean(1)
    m = h.max(1, keepdims=True)
    ls = np.log(np.exp(h - m).sum(1, keepdims=True))
    return (h - m - ls).astype(np.float32)


def _prep_core_inputs(x0t2, idx_tbl, r, nv, ns_pad, common):
    G = ns_pad // 128
    # shifted int16 table rows for this core's edges
    vals = np.zeros(ns_pad * DEG, np.int16)
    vals[:nv * DEG] = idx_tbl[r * nv * DEG:(r + 1) * nv * DEG]
    # edge (node m, slot k): m = g*128 + p.  Gather h covers slots 8h..8h+8;
    # its list position i maps to (p = i%128, j = i//128, k = 8h+j).
    e = vals.reshape(G, 128, DEG)                # [g, p, k]
    # the gather ucode trims TRAILING negative indices from each 1024-list
    # (doc: "negative indices at the end are ignored").  List position 1023
    # is (p=127, slot 7 or 15); edge order within a node is free (softmax is
    # slot-permutation invariant), so park non-negative idx there.
    for g in range(G):
        r127 = e[g, 127].copy()
        if r127[7] < 0 or r127[15] < 0:
            pos = np.where(r127 >= 0)[0]
            assert len(pos) >= 2, f"group {g}: node 127 lacks 2 idx>=0 edges"
            a, b = int(pos[0]), int(pos[1])
            rest = [s for s in range(DEG) if s not in (a, b)]
            order = rest[:7] + [a] + rest[7:] + [b]
            e[g, 127] = r127[order]
    idx = np.zeros((128, G * 128), np.int16)
    for g in range(G):
        for h in range(2):
            lst = e[g, :, 8 * h:8 * h + 8].T.reshape(-1)  # i = j*128 + p
            idx[:, g * 128 + h * 64:g * 128 + (h + 1) * 64] = np.tile(
                lst.reshape(64, 16).T, (8, 1))
    return dict(x0t=x0t2[r], idx=idx, **common)


def kernel(**inputs) -> np.ndarray:
    global LAST_RESULTS
    feats = np.ascontiguousarray(np.asarray(inputs["features"],
                                            dtype=np.float32))
    src = np.asarray(inputs["src"]).astype(np.int64).ravel()
    dst = np.asarray(inputs["dst"]).astype(np.int64).ravel()
    W1 = np.asarray(inputs["W1"], dtype=np.float32)
    al1 = np.asarray(inputs["al1"], dtype=np.float32)
    ar1 = np.asarray(inputs["ar1"], dtype=np.float32)
    b1 = np.asarray(inputs["b1"], dtype=np.float32)
    Wh = np.asarray(inputs["Wh"], dtype=np.float32)
    alh = np.asarray(inputs["alh"], dtype=np.float32)
    arh = np.asarray(inputs["arh"], dtype=np.float32)
    bh = np.asarray(inputs["bh"], dtype=np.float32)
    W2 = np.asarray(inputs["W2"], dtype=np.float32)
    al2 = np.asarray(inputs["al2"], dtype=np.float32)
    ar2 = np.asarray(inputs["ar2"], dtype=np.float32)
    b2 = np.asarray(inputs["b2"], dtype=np.float32)

    n = feats.shape[0]
    expected_dst = np.repeat(np.arange(N, dtype=np.int64), DEG)
    if (n != N or src.shape[0] != N * DEG
            or not np.array_equal(dst, expected_dst)
            or src.min() < 0 or src.max() >= N):
        return _numpy_fallback(feats, src, dst, W1, al1, ar1, b1,
                               Wh, alh, arh, bh, W2, al2, ar2, b2)

    from concourse.bass_utils import run_bass_kernel_spmd

    G = (NV + 127) // 128
    ns_pad = G * 128  # 6272
    NT = NCORES * ns_pad
    nc = _get_program(NCORES, ns_pad)

    # table row for node (core c, local n) under the 3-region collective
    # layout; stored shifted by NT/2 for the mid-table gather base.
    LRB = np.array([128 * b for b in GB], np.int64)   # [0,3200,5760,6272]
    SZ = np.diff(LRB)
    GFB = np.concatenate([[0], np.cumsum(NCORES * SZ)])
    core = src // NV
    local = src % NV
    reg = np.searchsorted(LRB, local, side="right") - 1
    row = GFB[reg] + core * SZ[reg] + (local - LRB[reg])
    idx_tbl = (row - SHIFT).astype(np.int16)

    # x^T packed as [128, 2, ns_pad]: x0t2[p, c*ns_pad+n] = x[n, c*128+p]
    xT16 = feats.T.astype(np.float16)                 # [IN, N]
    x0t2 = np.zeros((NCORES, 128, 2 * ns_pad), np.float16)
    for r in range(NCORES):
        blk = xT16[:, r * NV:(r + 1) * NV]            # [256, NV]
        x0t2[r, :, 0:NV] = blk[0:128]
        x0t2[r, :, ns_pad:ns_pad + NV] = blk[128:256]

    def bcast(a, w):
        return np.ascontiguousarray(
            np.broadcast_to(a.reshape(1, w), (128, w)).astype(np.float32))

    def ext(W, al, ar):
        Hh, Dd = al.shape
        Wr = W.reshape(W.shape[0], Hh, Dd)
        wal = np.einsum("khd,hd->kh", Wr, al).astype(np.float32)
        war = np.einsum("khd,hd->kh", Wr, ar).astype(np.float32)
        return np.ascontiguousarray(
            np.concatenate([W, wal, war], axis=1).astype(np.float16))

    def bias_ext(b, w):
        v = np.zeros(w, np.float32)
        v[:b.shape[0]] = b
        return bcast(v, w)

    common = dict(
        w1=ext(W1, al1, ar1), wh=ext(Wh, alh, arh), w2=ext(W2, al2, ar2),
        be1=bias_ext(b1, SUB12), beh=bias_ext(bh, HH + 1),
        be2=bias_ext(b2, SUB3),
    )
    in_maps = [
        _prep_core_inputs(x0t2, idx_tbl, r, NV, ns_pad, common)
        for r in range(NCORES)
    ]

    trace = bool(int(os.environ.get("GAT_TRACE", "0")))
    LAST_RESULTS = run_bass_kernel_spmd(
        nc, in_maps, list(range(NCORES)), trace=trace)
    outs = [LAST_RESULTS.results[r]["out"][:NV] for r in range(NCORES)]
    return np.ascontiguousarray(np.concatenate(outs, axis=0),
                                dtype=np.float32)


# revision 27
# speedup vs baseline: 1.5602x; 1.5602x over previous
"""Trainium2 Bass kernel: 3-layer GAT (nn_GAT_62182536511748).

Strategy (8 NeuronCores, SPMD, fp16 single-row gather), v3:
  - Nodes sharded contiguously across cores (6250 valid/core, padded to
    6272 = 49*128). dst == repeat(arange(N), 16): 16 in-edges per node.
  - Per layer each core computes feat = x_shard @ W (fp16 PE) plus the
    attention dots el/er via an extended weight matrix. Bias is folded
    into the table rows (softmax weights sum to 1, so
    sum_k alpha_k (feat_k + b) == out + b exactly).
  - Table rows are SINGLE-node [feat|el] fp16 at a 512B (L1/L2) / 256B
    (L3) stride. int16 gather indices only span +-32k, so indices are
    stored SHIFTED by NT/2 and the gather base points at the table
    middle - negative indices address the lower half (verified on HW).
  - The table is AllGather'd in three region chunks (fired early, with
    a 3-group lag so the gpsimd gather stream keeps its lookahead).
  - Edge phase per 128-node group: two 1024-index dma_gather
    instructions fetch one 264B/82B row per edge straight into
    node-slot position; fp16 multiply + fp16 pairwise add-tree
    aggregate; per-node softmax scale applied after the reduction.
  - The NEXT layer's feat phase (PE transpose + matmul) is interleaved
    into the current layer's edge loop; h never round-trips DRAM.
  - Scalar engine runs Exp only (leaky/relu/copies on DVE; plain
    tensor_scalar is avoided - it hits a slow ucode path - in favor of
    scalar_tensor_tensor). Final log-softmax Ln is one batched tail op.
"""

import os
import numpy as np

# ---- fixed problem dims -------------------------------------------------
N = 50000
DEG = 16
IN = 256
HID = 32
HEAD = 4
OUT = 40
HH = HID * HEAD  # 128
NEG_SLOPE = 0.2
NCORES = 8
NV = N // NCORES          # 6250 valid nodes per core

SUB12 = HH + 4            # 132 fp16 payload per node row (layers 1/2)
ROW12 = 256               # fp16 row stride (512B)
SUB3 = OUT + 1            # 41
ROW3 = 128                # 256B stride
NIDX = 1024               # indices per dma_gather (2048 crashes the ring)
GB = (0, 25, 45, 49)      # collective region boundaries in groups
LAG = 3                   # groups of slack before firing a collective
SHIFT = 17408             # gather base row: idx = row-SHIFT in [-17408,32767]

_PROGRAM_CACHE = {}
LAST_RESULTS = None


def _dma_gather_raw(nc, mybir, out_ap, in_ap, idxs_ap, num_idxs, elem_size,
                    elem_step, queue_num=0):
    """dma_gather minus the over-strict elem%256B assert (stride must still
    be a 256B multiple; verified on HW with 528B/264B/164B/82B elems)."""
    eng = nc.gpsimd
    stride_bytes = elem_step * mybir.dt.size(in_ap.dtype)
    assert stride_bytes % 256 == 0 and stride_bytes // 256 < 256
    _in_ap = eng.lower_ap_dma(in_ap, for_custom_bir_dma=True)
    _idxs_ap = eng.lower_ap(idxs_ap)
    _out_ap = eng.lower_ap(out_ap)
    return eng.add_instruction(
        mybir.InstDMAGatherAnt(
            name=nc.get_next_instruction_name(),
            ins=[*_in_ap, _idxs_ap,
                 eng.lower_val_access(eng.to_reg(num_idxs))],
            outs=[_out_ap],
            transpose=False, num_idxs=num_idxs, elem_size=elem_size,
            stride_bytes_256=stride_bytes // 256, gen_mode=0,
            single_packet=True, queue_num=queue_num,
            sbuf_tokens_per_rank=0, sbuf_free_dim_per_rank=0,
            sbuf_free_dim_pad_per_rank=0, sbuf_byte_offset=0,
        ))


# ========================================================================
# device program
# ========================================================================
def _build_program(ncores: int, ns_pad: int):
    from concourse import bass, mybir, tile, bacc
    from concourse.masks import make_identity
    from concourse.library_config import mlp

    f32 = mybir.dt.float32
    f16 = mybir.dt.float16
    i16 = mybir.dt.int16
    AX = mybir.AxisListType
    OPT = mybir.AluOpType
    AF = mybir.ActivationFunctionType

    G = ns_pad // 128
    NT = ncores * ns_pad          # table rows (50176)
    LRB = [128 * b for b in GB]   # local row bounds  [0,3200,5760,6272]
    GFB = [ncores * b for b in LRB]

    nc = bacc.Bacc(
        "TRN2", target_bir_lowering=False, debug=False,
        enable_asserts=False, num_devices=ncores, num_swdge_queues=4)

    # ---- kernel I/O ----
    x0t_d = nc.dram_tensor("x0t", [128, 2 * ns_pad], f16,
                           kind="ExternalInput").ap()
    idx_d = nc.dram_tensor("idx", [128, G * 128], i16,
                           kind="ExternalInput").ap()
    w1_d = nc.dram_tensor("w1", [IN, HH + 2 * HEAD], f16,
                          kind="ExternalInput").ap()
    wh_d = nc.dram_tensor("wh", [HH, HH + 2], f16, kind="ExternalInput").ap()
    w2_d = nc.dram_tensor("w2", [HH, OUT + 2], f16,
                          kind="ExternalInput").ap()
    be1_d = nc.dram_tensor("be1", [128, SUB12], f32, kind="ExternalInput").ap()
    beh_d = nc.dram_tensor("beh", [128, HH + 1], f32,
                           kind="ExternalInput").ap()
    be2_d = nc.dram_tensor("be2", [128, SUB3], f32, kind="ExternalInput").ap()
    out_d = nc.dram_tensor("out", [ns_pad, OUT], f32,
                           kind="ExternalOutput").ap()

    shared = "Shared" if ncores > 4 else "Local"
    gs1_d = nc.dram_tensor("gs1", [ns_pad, ROW12], f16).ap()
    gs2_d = nc.dram_tensor("gs2", [ns_pad, ROW12], f16).ap()
    gs3_d = nc.dram_tensor("gs3", [ns_pad, ROW3], f16).ap()
    gf1_d = nc.dram_tensor("gf1", [NT, ROW12], f16, addr_space=shared).ap()
    gf2_d = nc.dram_tensor("gf2", [NT, ROW12], f16, addr_space=shared).ap()
    gf3_d = nc.dram_tensor("gf3", [NT, ROW3], f16, addr_space=shared).ap()

    rgroups = [list(range(ncores))]

    with tile.TileContext(nc) as tc:
        with (
            tc.tile_pool(name="const", bufs=1) as cp,
            tc.tile_pool(name="feat", bufs=3) as fp,
            tc.tile_pool(name="edge", bufs=3) as ep,
            tc.tile_pool(name="psum", bufs=2, space="PSUM") as pp,
        ):
            nc.gpsimd.load_library(mlp)
            ident32 = cp.tile([128, 128], f32)
            make_identity(nc, ident32[:])
            ident16 = cp.tile([128, 128], f16)
            nc.vector.tensor_copy(ident16[:], ident32[:])
            zeros = cp.tile([128, HH], f32)
            nc.vector.memset(zeros[:], 0.0)
            idx_sb = cp.tile([128, G * 128], i16)
            nc.sync.dma_start(out=idx_sb[:], in_=idx_d[:, :])
            w1a = cp.tile([128, HH + 2 * HEAD], f16)
            w1b = cp.tile([128, HH + 2 * HEAD], f16)
            nc.sync.dma_start(out=w1a[:], in_=w1_d[0:128, :])
            nc.sync.dma_start(out=w1b[:], in_=w1_d[128:256, :])
            wh_sb = cp.tile([128, HH + 2], f16)
            nc.sync.dma_start(out=wh_sb[:], in_=wh_d[:, :])
            w2_sb = cp.tile([128, OUT + 2], f16)
            nc.sync.dma_start(out=w2_sb[:], in_=w2_d[:, :])
            be1 = cp.tile([128, SUB12], f32)
            nc.sync.dma_start(out=be1[:], in_=be1_d[:, :])
            beh = cp.tile([128, HH + 1], f32)
            nc.sync.dma_start(out=beh[:], in_=beh_d[:, :])
            be2 = cp.tile([128, SUB3], f32)
            nc.sync.dma_start(out=be2[:], in_=be2_d[:, :])
            er1 = cp.tile([128, G * HEAD], f32)
            er2 = cp.tile([128, G], f32)
            er3 = cp.tile([128, G], f32)
            ht3 = cp.tile([128, G * OUT], f32)
            nm3 = cp.tile([128, G], f32)
            s3 = cp.tile([128, G], f32)
            ls3 = cp.tile([128, G], f32)

            def region_cc(gs_d, gf_d, r):
                a, b = LRB[r], LRB[r + 1]
                nc.gpsimd.collective_compute(
                    "AllGather", OPT.bypass, replica_groups=rgroups,
                    ins=[gs_d[a:b, :]],
                    outs=[gf_d[GFB[r]:GFB[r + 1], :]])

            def maybe_cc(gs_d, gf_d, g, lags):
                for r in range(3):
                    if g == min(GB[r + 1] - 1 + lags[r], G - 1):
                        region_cc(gs_d, gf_d, r)

            def fence(gf_d, tag):
                """Order the gpsimd gather stream after all three region
                collectives: the raw gather only declares the upper half of
                the table as its input (the base is mid-table for signed
                idx), so collectives writing the lower half are otherwise
                unordered against it. Probe one row per region via sync-DMA,
                then touch the probe tile from gpsimd."""
                probe = cp.tile([128, ROW12], f16, name=f"probe_{tag}")
                for r in range(3):
                    w = min(ROW12, gf_d.shape[1])
                    nc.sync.dma_start(out=probe[r:r + 1, 0:w],
                                      in_=gf_d[GFB[r]:GFB[r] + 1, 0:w])
                scr = cp.tile([128, 8], f16, name=f"scr_{tag}")
                nc.gpsimd.tensor_copy(scr[0:3, :], probe[0:3, 0:8])

            def gathers(gf_d, sub, row, g):
                big = ep.tile([128, DEG * sub], f16, tag=f"big{row}", bufs=8,
                              name=f"big_{row}_{g}")
                for h in range(2):
                    _dma_gather_raw(
                        nc, mybir,
                        big[:, h * 8 * sub:(h + 1) * 8 * sub],
                        gf_d[SHIFT:NT, 0:sub],
                        idx_sb[:, g * 128 + h * 64:g * 128 + (h + 1) * 64],
                        NIDX, sub, row, queue_num=(2 * g + h) % 4)
                return big[:].rearrange("p (k r) -> p k r", r=sub)

            # ---------------- L1 feat ----------------
            for g in range(G):
                c0, c1 = g * 128, (g + 1) * 128
                xt = fp.tile([128, 256], f16, tag="xt", bufs=4,
                             name=f"A_x{g}")
                xv = x0t_d[:, :].rearrange("p (c n) -> p c n", n=ns_pad)
                nc.sync.dma_start(
                    out=xt[:].rearrange("p (c n) -> p c n", c=2),
                    in_=xv[:, :, c0:c1])
                fps = pp.tile([128, HH + 2 * HEAD], f32, tag="fps1",
                              name=f"A_fps{g}")
                nc.tensor.matmul(fps[:], lhsT=xt[:, 0:128], rhs=w1a[:],
                                 start=True, stop=False)
                nc.tensor.matmul(fps[:], lhsT=xt[:, 128:256], rhs=w1b[:],
                                 start=False, stop=True)
                grow = fp.tile([128, SUB12], f16, tag="grow",
                               name=f"A_grow{g}")
                nc.vector.tensor_tensor(out=grow[:], in0=fps[:, 0:SUB12],
                                        in1=be1[:], op=OPT.add)
                nc.vector.tensor_copy(er1[:, g * HEAD:(g + 1) * HEAD],
                                      fps[:, SUB12:SUB12 + HEAD])
                nc.sync.dma_start(out=gs1_d[c0:c1, 0:SUB12], in_=grow[:])
                maybe_cc(gs1_d, gf1_d, g, (0, 0, 0))

            # ---------------- L1 edge + L2 feat ----------------
            fence(gf1_d, "1")
            for g in range(G):
                bv = gathers(gf1_d, SUB12, ROW12, g)
                feat_e = bv[:, :, 0:HH]              # [128,16,128] f16
                el_e = bv[:, :, HH:HH + HEAD]        # [128,16,4] f16
                f_all = ep.tile([128, DEG * HH], f16, tag="f_all",
                                name=f"B_fa{g}")
                e_t = ep.tile([128, DEG * HEAD], f32, tag="e_t",
                              name=f"B_et{g}")
                erv = (er1[:, g * HEAD:(g + 1) * HEAD]
                       .unsqueeze(1).to_broadcast((128, DEG, HEAD)))
                nc.vector.tensor_tensor(
                    out=e_t[:].rearrange("p (k h) -> p k h", h=HEAD),
                    in0=el_e, in1=erv, op=OPT.add)
                e2 = ep.tile([128, DEG * HEAD], f32, tag="e2",
                             name=f"B_e2{g}")
                nc.vector.scalar_tensor_tensor(
                    out=e2[:], in0=e_t[:], scalar=NEG_SLOPE, in1=e_t[:],
                    op0=OPT.mult, op1=OPT.max)
                ex = ep.tile([128, DEG * HEAD], f16, tag="ex",
                             name=f"B_ex{g}")
                nc.scalar.activation(out=ex[:], in_=e2[:], func=AF.Exp)
                den = ep.tile([128, HEAD], f32, tag="den", name=f"B_den{g}")
                nc.vector.tensor_reduce(
                    out=den[:],
                    in_=ex[:].rearrange("p (k h) -> p h k", h=HEAD),
                    axis=AX.X, op=OPT.add)
                inv = ep.tile([128, HEAD], f32, tag="inv", name=f"B_inv{g}")
                nc.vector.reciprocal(inv[:], den[:])
                featv = feat_e.rearrange("p k (h d) -> p k h d", h=HEAD)
                exv = (ex[:].rearrange("p (k h) -> p k h", h=HEAD)
                       .unsqueeze(3).to_broadcast((128, DEG, HEAD, HID)))
                nc.vector.tensor_tensor(
                    out=f_all[:].rearrange("p (k h d) -> p k h d",
                                           k=DEG, h=HEAD),
                    in0=featv, in1=exv, op=OPT.mult)
                for wdt in (8 * HH, 4 * HH, 2 * HH):
                    nc.vector.tensor_tensor(
                        out=f_all[:, 0:wdt], in0=f_all[:, 0:wdt],
                        in1=f_all[:, wdt:2 * wdt], op=OPT.add)
                u = ep.tile([128, HH], f32, tag="u", name=f"B_u{g}")
                nc.vector.tensor_tensor(out=u[:], in0=f_all[:, 0:HH],
                                        in1=f_all[:, HH:2 * HH], op=OPT.add)
                t1 = ep.tile([128, HH], f32, tag="t1", name=f"B_t1{g}")
                invv = inv[:].unsqueeze(2).to_broadcast((128, HEAD, HID))
                nc.vector.tensor_tensor(
                    out=t1[:].rearrange("p (h d) -> p h d", h=HEAD),
                    in0=u[:].rearrange("p (h d) -> p h d", h=HEAD),
                    in1=invv, op=OPT.mult)
                h1 = ep.tile([128, HH], f16, tag="h1", name=f"B_h1{g}")
                nc.vector.scalar_tensor_tensor(
                    out=h1[:], in0=t1[:], scalar=0.0, in1=zeros[:],
                    op0=OPT.max, op1=OPT.max)
                # ---- L2 feat for this group ----
                hT_ps = pp.tile([128, 128], f16, tag="hT", name=f"B_hT{g}")
                nc.tensor.transpose(hT_ps[:], h1[:], ident16[:])
                hT = fp.tile([128, 128], f16, tag="hTs", name=f"B_hTs{g}")
                nc.vector.tensor_copy(hT[:], hT_ps[:])
                fps2 = pp.tile([128, HH + 2], f32, tag="fps2",
                               name=f"B_fps2{g}")
                nc.tensor.matmul(fps2[:], lhsT=hT[:], rhs=wh_sb[:],
                                 start=True, stop=True)
                grow2 = fp.tile([128, HH + 1], f16, tag="grow",
                                name=f"B_grow2{g}")
                nc.vector.tensor_tensor(out=grow2[:],
                                        in0=fps2[:, 0:HH + 1],
                                        in1=beh[:], op=OPT.add)
                nc.vector.tensor_copy(er2[:, g:g + 1],
                                      fps2[:, HH + 1:HH + 2])
                nc.sync.dma_start(out=gs2_d[g * 128:(g + 1) * 128, 0:HH + 1],
                                  in_=grow2[:])
                maybe_cc(gs2_d, gf2_d, g, (LAG, 1, 0))

            # ---------------- L2 edge + L3 feat ----------------
            fence(gf2_d, "2")
            for g in range(G):
                bv = gathers(gf2_d, SUB12, ROW12, g)
                feat_e = bv[:, :, 0:HH]
                el_e = bv[:, :, HH:HH + 1].rearrange("p k o -> p (k o)")
                f_all = ep.tile([128, DEG * HH], f16, tag="f_all",
                                name=f"C_fa{g}")
                e_t = ep.tile([128, DEG], f32, tag="e_t", name=f"C_et{g}")
                nc.vector.scalar_tensor_tensor(
                    out=e_t[:], in0=el_e, scalar=er2[:, g:g + 1],
                    in1=zeros[:, 0:DEG], op0=OPT.add, op1=OPT.add)
                e2 = ep.tile([128, DEG], f32, tag="e2", name=f"C_e2{g}")
                nc.vector.scalar_tensor_tensor(
                    out=e2[:], in0=e_t[:], scalar=NEG_SLOPE, in1=e_t[:],
                    op0=OPT.mult, op1=OPT.max)
                ex = ep.tile([128, DEG], f16, tag="ex", name=f"C_ex{g}")
                den = ep.tile([128, 1], f32, tag="den", name=f"C_den{g}")
                nc.scalar.activation(out=ex[:], in_=e2[:], func=AF.Exp,
                                     accum_out=den[:])
                inv = ep.tile([128, 1], f32, tag="inv", name=f"C_inv{g}")
                nc.vector.reciprocal(inv[:], den[:])
                exv = ex[:].unsqueeze(2).to_broadcast((128, DEG, HH))
                nc.vector.tensor_tensor(
                    out=f_all[:].rearrange("p (k d) -> p k d", k=DEG),
                    in0=feat_e, in1=exv, op=OPT.mult)
                for wdt in (8 * HH, 4 * HH, 2 * HH):
                    nc.vector.tensor_tensor(
                        out=f_all[:, 0:wdt], in0=f_all[:, 0:wdt],
                        in1=f_all[:, wdt:2 * wdt], op=OPT.add)
                u = ep.tile([128, HH], f32, tag="u", name=f"C_u{g}")
                nc.vector.tensor_tensor(out=u[:], in0=f_all[:, 0:HH],
                                        in1=f_all[:, HH:2 * HH], op=OPT.add)
                h2 = ep.tile([128, HH], f16, tag="h1", name=f"C_h2{g}")
                nc.vector.scalar_tensor_tensor(
                    out=h2[:], in0=u[:], scalar=inv[:, 0:1], in1=zeros[:],
                    op0=OPT.mult, op1=OPT.max)
                # ---- L3 feat for this group ----
                hT_ps = pp.tile([128, 128], f16, tag="hT", name=f"C_hT{g}")
                nc.tensor.transpose(hT_ps[:], h2[:], ident16[:])
                hT = fp.tile([128, 128], f16, tag="hTs", name=f"C_hTs{g}")
                nc.vector.tensor_copy(hT[:], hT_ps[:])
                fps3 = pp.tile([128, OUT + 2], f32, tag="fps3",
                               name=f"C_fps3{g}")
                nc.tensor.matmul(fps3[:], lhsT=hT[:], rhs=w2_sb[:],
                                 start=True, stop=True)
                grow3 = fp.tile([128, SUB3], f16, tag="grow3",
                                name=f"C_grow3{g}")
                nc.vector.tensor_tensor(out=grow3[:], in0=fps3[:, 0:SUB3],
                                        in1=be2[:], op=OPT.add)
                nc.vector.tensor_copy(er3[:, g:g + 1],
                                      fps3[:, SUB3:SUB3 + 1])
                nc.sync.dma_start(out=gs3_d[g * 128:(g + 1) * 128, 0:SUB3],
                                  in_=grow3[:])
                maybe_cc(gs3_d, gf3_d, g, (LAG, 1, 0))

            # ---------------- L3 edge ----------------
            fence(gf3_d, "3")
            for g in range(G):
                bv = gathers(gf3_d, SUB3, ROW3, g)
                feat_e = bv[:, :, 0:OUT]
                el_e = bv[:, :, OUT:OUT + 1].rearrange("p k o -> p (k o)")
                e_t = ep.tile([128, DEG], f32, tag="e_t", name=f"D_et{g}")
                nc.vector.scalar_tensor_tensor(
                    out=e_t[:], in0=el_e, scalar=er3[:, g:g + 1],
                    in1=zeros[:, 0:DEG], op0=OPT.add, op1=OPT.add)
                e2 = ep.tile([128, DEG], f32, tag="e2", name=f"D_e2{g}")
                nc.vector.scalar_tensor_tensor(
                    out=e2[:], in0=e_t[:], scalar=NEG_SLOPE, in1=e_t[:],
                    op0=OPT.mult, op1=OPT.max)
                ex = ep.tile([128, DEG], f16, tag="ex", name=f"D_ex{g}")
                den = ep.tile([128, 1], f32, tag="den", name=f"D_den{g}")
                nc.scalar.activation(out=ex[:], in_=e2[:], func=AF.Exp,
                                     accum_out=den[:])
                inv = ep.tile([128, 1], f32, tag="inv", name=f"D_inv{g}")
                nc.vector.reciprocal(inv[:], den[:])
                f_all = ep.tile([128, DEG * OUT], f16, tag="fa3",
                                name=f"D_fa{g}")
                exv = ex[:].unsqueeze(2).to_broadcast((128, DEG, OUT))
                nc.vector.tensor_tensor(
                    out=f_all[:].rearrange("p (k d) -> p k d", k=DEG),
                    in0=feat_e, in1=exv, op=OPT.mult)
                for wdt in (8 * OUT, 4 * OUT, 2 * OUT):
                    nc.vector.tensor_tensor(
                        out=f_all[:, 0:wdt], in0=f_all[:, 0:wdt],
                        in1=f_all[:, wdt:2 * wdt], op=OPT.add)
                u = ep.tile([128, OUT], f32, tag="u3", name=f"D_u{g}")
                nc.vector.tensor_tensor(out=u[:], in0=f_all[:, 0:OUT],
                                        in1=f_all[:, OUT:2 * OUT], op=OPT.add)
                htc = ht3[:, g * OUT:(g + 1) * OUT]
                nc.vector.scalar_tensor_tensor(
                    out=htc, in0=u[:], scalar=inv[:, 0:1],
                    in1=zeros[:, 0:OUT], op0=OPT.mult, op1=OPT.add)
                nc.vector.reduce_max(out=nm3[:, g:g + 1], in_=htc,
                                     axis=AX.X, negate=True)
                exf = ep.tile([128, OUT], f16, tag="exf", name=f"D_exf{g}")
                nc.scalar.activation(out=exf[:], in_=htc, func=AF.Exp,
                                     bias=nm3[:, g:g + 1],
                                     accum_out=s3[:, g:g + 1])

            # ---------------- log-softmax tail ----------------
            nc.scalar.activation(out=ls3[:], in_=s3[:], func=AF.Ln)
            for g in range(G):
                o_t = ep.tile([128, OUT], f32, tag="o_t", name=f"E_o{g}")
                nc.vector.scalar_tensor_tensor(
                    out=o_t[:], in0=ht3[:, g * OUT:(g + 1) * OUT],
                    scalar=nm3[:, g:g + 1],
                    in1=ls3[:, g:g + 1].to_broadcast((128, OUT)),
                    op0=OPT.add, op1=OPT.subtract)
                nc.sync.dma_start(out=out_d[g * 128:(g + 1) * 128, :],
                                  in_=o_t[:])

    nc.compile()
    return nc


# ========================================================================
# host side
# ========================================================================
def _get_program(ncores, ns_pad):
    key = (ncores, ns_pad)
    if key not in _PROGRAM_CACHE:
        _PROGRAM_CACHE[key] = _build_program(ncores, ns_pad)
    return _PROGRAM_CACHE[key]


def _numpy_fallback(feats, src, dst, W1, al1, ar1, b1, Wh, alh, arh, bh,
                    W2, al2, ar2, b2):
    n = feats.shape[0]

    def gat(x, W, al, ar, b):
        Hh, Dd = al.shape
        feat = (x @ W).reshape(n, Hh, Dd)
        el = (feat * al).sum(-1)
        er = (feat * ar).sum(-1)
        e = el[src] + er[dst]
        e = np.where(e > 0, e, NEG_SLOPE * e).astype(np.float32)
        emax = np.full((n, Hh), -np.inf, np.float32)
        np.maximum.at(emax, dst, e)
        ex = np.exp(e - emax[dst])
        den = np.zeros((n, Hh), np.float32)
        np.add.at(den, dst, ex)
        alpha = ex / den[dst]
        out = np.zeros((n, Hh, Dd), np.float32)
        np.add.at(out, dst, feat[src] * alpha[..., None])
        return out + b.reshape(1, Hh, Dd)

    h = np.maximum(gat(feats, W1, al1, ar1, b1).reshape(n, HH), 0.0)
    h = np.maximum(gat(h, Wh, alh, arh, bh).mean(1), 0.0)
    h = gat(h, W2, al2, ar2, b2).mean(1)
    m = h.max(1, keepdims=True)
    ls = np.log(np.exp(h - m).sum(1, keepdims=True))
    return (h - m - ls).astype(np.float32)


def _prep_core_inputs(x0t2, idx_tbl, r, nv, ns_pad, common):
    G = ns_pad // 128
    # shifted int16 table rows for this core's edges
    vals = np.zeros(ns_pad * DEG, np.int16)
    vals[:nv * DEG] = idx_tbl[r * nv * DEG:(r + 1) * nv * DEG]
    # edge (node m, slot k): m = g*128 + p.  Gather h covers slots 8h..8h+8;
    # its list position i maps to (p = i%128, j = i//128, k = 8h+j).
    e = vals.reshape(G, 128, DEG)                # [g, p, k]
    # the gather ucode trims TRAILING negative indices from each 1024-list
    # (doc: "negative indices at the end are ignored").  List position 1023
    # is (p=127, slot 7 or 15); edge order within a node is free (softmax is
    # slot-permutation invariant), so park non-negative idx there.
    for g in range(G):
        r127 = e[g, 127].copy()
        if r127[7] < 0 or r127[15] < 0:
            pos = np.where(r127 >= 0)[0]
            assert len(pos) >= 2, f"group {g}: node 127 lacks 2 idx>=0 edges"
            a, b = int(pos[0]), int(pos[1])
            rest = [s for s in range(DEG) if s not in (a, b)]
            order = rest[:7] + [a] + rest[7:] + [b]
            e[g, 127] = r127[order]
    idx = np.zeros((128, G * 128), np.int16)
    for g in range(G):
        for h in range(2):
            lst = e[g, :, 8 * h:8 * h + 8].T.reshape(-1)  # i = j*128 + p
            idx[:, g * 128 + h * 64:g * 128 + (h + 1) * 64] = np.tile(
                lst.reshape(64, 16).T, (8, 1))
    return dict(x0t=x0t2[r], idx=idx, **common)


def kernel(**inputs) -> np.ndarray:
    global LAST_RESULTS
    feats = np.ascontiguousarray(np.asarray(inputs["features"],
                                            dtype=np.float32))
    src = np.asarray(inputs["src"]).astype(np.int64).ravel()
    dst = np.asarray(inputs["dst"]).astype(np.int64).ravel()
    W1 = np.asarray(inputs["W1"], dtype=np.float32)
    al1 = np.asarray(inputs["al1"], dtype=np.float32)
    ar1 = np.asarray(inputs["ar1"], dtype=np.float32)
    b1 = np.asarray(inputs["b1"], dtype=np.float32)
    Wh = np.asarray(inputs["Wh"], dtype=np.float32)
    alh = np.asarray(inputs["alh"], dtype=np.float32)
    arh = np.asarray(inputs["arh"], dtype=np.float32)
    bh = np.asarray(inputs["bh"], dtype=np.float32)
    W2 = np.asarray(inputs["W2"], dtype=np.float32)
    al2 = np.asarray(inputs["al2"], dtype=np.float32)
    ar2 = np.asarray(inputs["ar2"], dtype=np.float32)
    b2 = np.asarray(inputs["b2"], dtype=np.float32)

    n = feats.shape[0]
    expected_dst = np.repeat(np.arange(N, dtype=np.int64), DEG)
    if (n != N or src.shape[0] != N * DEG
            or not np.array_equal(dst, expected_dst)
            or src.min() < 0 or src.max() >= N):
        return _numpy_fallback(feats, src, dst, W1, al1, ar1, b1,
                               Wh, alh, arh, bh, W2, al2, ar2, b2)

    from concourse.bass_utils import run_bass_kernel_spmd

    G = (NV + 127) // 128
    ns_pad = G * 128  # 6272
    NT = NCORES * ns_pad
    nc = _get_program(NCORES, ns_pad)

    # table row for node (core c, local n) under the 3-region collective
    # layout; stored shifted by NT/2 for the mid-table gather base.
    LRB = np.array([128 * b for b in GB], np.int64)   # [0,3200,5760,6272]
    SZ = np.diff(LRB)
    GFB = np.concatenate([[0], np.cumsum(NCORES * SZ)])
    core = src // NV
    local = src % NV
    reg = np.searchsorted(LRB, local, side="right") - 1
    row = GFB[reg] + core * SZ[reg] + (local - LRB[reg])
    idx_tbl = (row - SHIFT).astype(np.int16)

    # x^T packed as [128, 2, ns_pad]: x0t2[p, c*ns_pad+n] = x[n, c*128+p]
    xT16 = feats.T.astype(np.float16)                 # [IN, N]
    x0t2 = np.zeros((NCORES, 128, 2 * ns_pad), np.float16)
    for r in range(NCORES):
        blk = xT16[:, r * NV:(r + 1) * NV]            # [256, NV]
        x0t2[r, :, 0:NV] = blk[0:128]
        x0t2[r, :, ns_pad:ns_pad + NV] = blk[128:256]

    def bcast(a, w):
        return np.ascontiguousarray(
            np.broadcast_to(a.reshape(1, w), (128, w)).astype(np.float32))

    def ext(W, al, ar):
        Hh, Dd = al.shape
        Wr = W.reshape(W.shape[0], Hh, Dd)
        wal = np.einsum("khd,hd->kh", Wr, al).astype(np.float32)
        war = np.einsum("khd,hd->kh", Wr, ar).astype(np.float32)
        return np.ascontiguousarray(
            np.concatenate([W, wal, war], axis=1).astype(np.float16))

    def bias_ext(b, w):
        v = np.zeros(w, np.float32)
        v[:b.shape[0]] = b
        return bcast(v, w)

    common = dict(
        w1=ext(W1, al1, ar1), wh=ext(Wh, alh, arh), w2=ext(W2, al2, ar2),
        be1=bias_ext(b1, SUB12), beh=bias_ext(bh, HH + 1),
        be2=bias_ext(b2, SUB3),
    )
    in_maps = [
        _prep_core_inputs(x0t2, idx_tbl, r, NV, ns_pad, common)
        for r in range(NCORES)
    ]

    trace = bool(int(os.environ.get("GAT_TRACE", "0")))
    LAST_RESULTS = run_bass_kernel_spmd(
        nc, in_maps, list(range(NCORES)), trace=trace)
    outs = [LAST_RESULTS.results[r]["out"][:NV] for r in range(NCORES)]
    return np.ascontiguousarray(np.concatenate(outs, axis=0),
                                dtype=np.float32)
